# revision 5
# baseline (speedup 1.0000x reference)
"""LDDMM variational shooting RHS on 8 Trainium2 NeuronCores.

reference math (B=1, N=8192, D=3, sigma=0.1):
    p   = clip(mom, -1, 1)
    d2  = |x_i - x_j|^2
    K   = exp(-d2 / (2 sig^2)) = exp(-50 d2)
    dcp = K @ p
    W   = K * (p p^T)
    row = W @ 1;  Wx = W @ x
    dmom = (1/sig^2) (x * row - Wx)

Device strategy (row-sharded over 8 cores, 1024 rows each):
  - work in transposed tiles Kt[j, i] (j on SBUF partitions) so both the
    d2 generation and the j-contraction map onto the tensor engine.
  - d2 gen: single K_dim=13 fp16 matmul per [128j x 512i] tile using hi/lo
    split operands (fp16 streams 1 cycle/row; fp32 would cost 4x):
      d2[j,i] = sq_j + sq_i - 2(xh_j xh_i + xl_j xh_i + xh_j xl_i)
  - exp on the scalar (ACT) engine, PSUM -> SBUF fp16, grouped 3 tiles per
    instruction to amortize the per-instruction overhead.
  - everything downstream of K is one accumulating matmul with
      R = [p | vec(p (x) x)]  in R^{N x 12}:   S[m, i] = sum_j Kt[j,i] R[j,m]
    because  dcp_i = S[0:3, i],  row_i = p_i . dcp_i,
             (W x)_ie = sum_d p_id S[3+3d+e, i].
  - tiny host postprocess of S -> (dmom, dcp).
"""

import os
import sys

import numpy as np

if "/opt/trn_rl_repo" not in sys.path:
    sys.path.insert(0, "/opt/trn_rl_repo")

SIG2 = 0.01
N = 8192
D = 3
NCORES = 8
RPC = N // NCORES          # rows (i) per core = 1024
ICHUNK = 512               # i columns per matmul (one PSUM bank, fp32 out)
NIB = RPC // ICHUNK        # i-chunks per core = 2
JTILE = 128                # j rows per tile (PE contraction dim)
NJT = N // JTILE           # 64 j-tiles
GROUP = 3                  # j-tiles per ACT instruction (3 PSUM banks)
KDIM = 13                  # gen matmul contraction dim (hi/lo split)
RCOLS = 12                 # reduction matrix columns

_cache: dict = {}

# last BassKernelResults (exec_time_ns etc.) for the test harness
last_result = None


def _build_program(loop_m: int = 1):
    """Build (once) the Bass/Tile program shared by all 8 cores.

    loop_m > 1 unrolls the whole computation M times inside one NEFF —
    used only by the benchmarking harness to measure steady-state
    per-iteration device time through the axon dispatch overhead.
    """
    import concourse.bass as bass  # noqa: F401
    import concourse.mybir as mybir
    import concourse.tile as tile
    from concourse import bacc

    dt = mybir.dt
    nc = bacc.Bacc("TRN2", target_bir_lowering=False, debug=False)

    Ah = nc.dram_tensor("a_gen", [KDIM, N], dt.float16, kind="ExternalInput")
    Bh = nc.dram_tensor("b_gen", [KDIM, RPC], dt.float16, kind="ExternalInput")
    Rh = nc.dram_tensor("r_red", [JTILE, NJT * RCOLS], dt.float16,
                        kind="ExternalInput")
    So = nc.dram_tensor("s_out", [RCOLS, RPC], dt.float32,
                        kind="ExternalOutput")

    groups = []
    jt = 0
    while jt < NJT:
        groups.append(list(range(jt, min(jt + GROUP, NJT))))
        jt += GROUP

    with tile.TileContext(nc) as tc:
        with (
            tc.tile_pool(name="const", bufs=1) as cpool,
            tc.tile_pool(name="ksb", bufs=3) as kpool,
            tc.tile_pool(name="ssb", bufs=2) as spool,
            tc.tile_pool(name="d2", bufs=2, space="PSUM") as d2pool,
            tc.tile_pool(name="sacc", bufs=2, space="PSUM") as sapool,
        ):
            a_sb = cpool.tile([KDIM, N], dt.float16)
            b_sb = cpool.tile([KDIM, RPC], dt.float16)
            r_sb = cpool.tile([JTILE, NJT * RCOLS], dt.float16)
            nc.sync.dma_start(out=a_sb, in_=Ah.ap())
            nc.sync.dma_start(out=b_sb, in_=Bh.ap())
            nc.sync.dma_start(out=r_sb, in_=Rh.ap())

            for ib in [i % NIB for i in range(NIB * loop_m)]:
                s_ps = sapool.tile([RCOLS, ICHUNK], dt.float32)
                bcols = b_sb[:, ib * ICHUNK:(ib + 1) * ICHUNK]

                pending = None  # (jts, k_sb) whose reduction is not yet emitted
                for jts in groups:
                    w = len(jts) * ICHUNK
                    d2 = d2pool.tile([JTILE, GROUP * ICHUNK], dt.float32)
                    for idx, jt in enumerate(jts):
                        nc.tensor.matmul(
                            d2[:, idx * ICHUNK:(idx + 1) * ICHUNK],
                            a_sb[:, jt * JTILE:(jt + 1) * JTILE],
                            bcols,
                            start=True, stop=True,
                        )
                    # software pipeline: emit previous group's reductions
                    # between this group's gen and exp so the PE never FIFO
                    # blocks behind a reduction waiting on the ACT engine.
                    if pending is not None:
                        pjts, pk = pending
                        for idx, jt in enumerate(pjts):
                            nc.tensor.matmul(
                                s_ps,
                                r_sb[:, jt * RCOLS:(jt + 1) * RCOLS],
                                pk[:, idx * ICHUNK:(idx + 1) * ICHUNK],
                                start=(jt == 0), stop=(jt == NJT - 1),
                            )
                    k_sb = kpool.tile([JTILE, GROUP * ICHUNK], dt.float16)
                    nc.scalar.activation(
                        k_sb[:, :w], d2[:, :w],
                        mybir.ActivationFunctionType.Exp,
                        scale=-1.0 / (2.0 * SIG2),
                    )
                    pending = (jts, k_sb)

                pjts, pk = pending
                for idx, jt in enumerate(pjts):
                    nc.tensor.matmul(
                        s_ps,
                        r_sb[:, jt * RCOLS:(jt + 1) * RCOLS],
                        pk[:, idx * ICHUNK:(idx + 1) * ICHUNK],
                        start=(jt == 0), stop=(jt == NJT - 1),
                    )

                s_out = spool.tile([RCOLS, ICHUNK], dt.float32)
                nc.vector.tensor_copy(s_out, s_ps)
                nc.sync.dma_start(
                    out=So.ap()[:, ib * ICHUNK:(ib + 1) * ICHUNK], in_=s_out
                )

    nc.compile()
    return nc


def _split_hi_lo(v32: np.ndarray):
    """fp32 -> (hi, lo) float16 pair with v ~= hi + lo."""
    hi = v32.astype(np.float16)
    lo = (v32 - hi.astype(np.float32)).astype(np.float16)
    return hi, lo


def _host_prep(mom: np.ndarray, control_points: np.ndarray):
    x = np.asarray(control_points, np.float32).reshape(N, D)
    p = np.clip(np.asarray(mom, np.float32).reshape(N, D), -1.0, 1.0)

    sq = np.sum(x.astype(np.float64) * x.astype(np.float64), axis=1)
    sq = sq.astype(np.float32)
    xh, xl = _split_hi_lo(x)
    sqh, sql = _split_hi_lo(sq)
    ones = np.ones(N, np.float16)

    # lhsT (stationary, per-j): 13 rows
    A = np.empty((KDIM, N), np.float16)
    A[0:3] = xh.T
    A[3:6] = xl.T
    A[6:9] = xh.T
    A[9] = sqh
    A[10] = sql
    A[11] = ones
    A[12] = ones

    # rhs (moving, per-i): 13 rows
    m2xh = (-2.0 * xh.astype(np.float32)).astype(np.float16)
    m2xl = (-2.0 * xl.astype(np.float32)).astype(np.float16)
    Bfull = np.empty((KDIM, N), np.float16)
    Bfull[0:3] = m2xh.T
    Bfull[3:6] = m2xh.T
    Bfull[6:9] = m2xl.T
    Bfull[9] = ones
    Bfull[10] = ones
    Bfull[11] = sqh
    Bfull[12] = sql

    # reduction matrix R = [p | vec(p (x) x)], packed [128, 64*12]
    R = np.empty((N, RCOLS), np.float32)
    R[:, 0:3] = p
    R[:, 3:12] = (p[:, :, None] * x[:, None, :]).reshape(N, 9)
    Rp = (
        R.reshape(NJT, JTILE, RCOLS)
        .transpose(1, 0, 2)
        .reshape(JTILE, NJT * RCOLS)
        .astype(np.float16)
    )
    return x, p, A, Bfull, Rp


def kernel(mom: np.ndarray, control_points: np.ndarray):
    global last_result
    from concourse.bass_utils import run_bass_kernel_spmd

    x, p, A, Bfull, Rp = _host_prep(mom, control_points)

    loop_m = int(os.environ.get("KERNEL_LOOP_M", "1"))
    key = ("nc", loop_m)
    if key not in _cache:
        _cache[key] = _build_program(loop_m)
    nc = _cache[key]

    in_maps = []
    for c in range(NCORES):
        in_maps.append({
            "a_gen": A,
            "b_gen": np.ascontiguousarray(Bfull[:, c * RPC:(c + 1) * RPC]),
            "r_red": Rp,
        })

    trace = os.environ.get("KERNEL_TRACE", "0") == "1"
    res = run_bass_kernel_spmd(
        nc, in_maps, core_ids=list(range(NCORES)), trace=trace,
    )
    last_result = res

    S = np.concatenate([r["s_out"] for r in res.results], axis=1)  # [12, N]

    dcp = S[0:3].T                                   # [N, 3]
    row = np.einsum("nd,dn->n", p, S[0:3])           # p_i . (K p)_i
    Wx = np.einsum("nd,den->ne", p, S[3:12].reshape(D, D, N))
    dmom = (1.0 / SIG2) * (x * row[:, None] - Wx)

    return (
        dmom.reshape(1, N, D).astype(np.float32),
        dcp.reshape(1, N, D).astype(np.float32),
    )


# revision 7
# speedup vs baseline: 2.5937x; 2.5937x over previous
"""LDDMM variational shooting RHS on 8 Trainium2 NeuronCores.

reference math (B=1, N=8192, D=3, sigma=0.1):
    p   = clip(mom, -1, 1)
    d2  = |x_i - x_j|^2
    K   = exp(-d2 / (2 sig^2)) = exp(-50 d2)
    dcp = K @ p
    W   = K * (p p^T)
    row = W @ 1;  Wx = W @ x
    dmom = (1/sig^2) (x * row - Wx)

Device strategy (row-sharded over 8 cores, 1024 rows each):
  - work in transposed tiles Kt[j, i] (j on SBUF partitions) so both the
    d2 generation and the j-contraction map onto the tensor engine.
  - d2 gen: single K_dim=13 fp16 matmul per [128j x 512i] tile using hi/lo
    split operands (fp16 streams 1 cycle/row; fp32 would cost 4x):
      d2[j,i] = sq_j + sq_i - 2(xh_j xh_i + xl_j xh_i + xh_j xl_i)
  - exp on the scalar (ACT) engine, PSUM -> SBUF fp16, grouped 3 tiles per
    instruction to amortize the per-instruction overhead.
  - everything downstream of K is one accumulating matmul with
      R = [p | vec(p (x) x)]  in R^{N x 12}:   S[m, i] = sum_j Kt[j,i] R[j,m]
    because  dcp_i = S[0:3, i],  row_i = p_i . dcp_i,
             (W x)_ie = sum_d p_id S[3+3d+e, i].
  - tiny host postprocess of S -> (dmom, dcp).
"""

import os
import sys

import numpy as np

if "/opt/trn_rl_repo" not in sys.path:
    sys.path.insert(0, "/opt/trn_rl_repo")

SIG2 = 0.01
N = 8192
D = 3
NCORES = 8
RPC = N // NCORES          # rows (i) per core = 1024
ICHUNK = 512               # i columns per matmul (one PSUM bank, fp32 out)
NIB = RPC // ICHUNK        # i-chunks per core = 2
JTILE = 128                # j rows per tile (PE contraction dim)
NJT = N // JTILE           # 64 j-tiles
GROUP = 3                  # j-tiles per ACT instruction (3 PSUM banks)
KDIM = 13                  # gen matmul contraction dim (hi/lo split)
RCOLS = 12                 # reduction matrix columns

_cache: dict = {}

# benchmarking/sim-only ablation switch: "full" | "genact" | "gen"
VARIANT = "full"

# last BassKernelResults (exec_time_ns etc.) for the test harness
last_result = None


def _build_program(loop_m: int = 1):
    """Build (once) the Bass/Tile program shared by all 8 cores.

    loop_m > 1 unrolls the whole computation M times inside one NEFF —
    used only by the benchmarking harness to measure steady-state
    per-iteration device time through the axon dispatch overhead.
    """
    import concourse.bass as bass  # noqa: F401
    import concourse.mybir as mybir
    import concourse.tile as tile
    from concourse import bacc

    dt = mybir.dt
    nc = bacc.Bacc("TRN2", target_bir_lowering=False, debug=False)

    Ah = nc.dram_tensor("a_gen", [KDIM, N], dt.float16, kind="ExternalInput")
    Bh = nc.dram_tensor("b_gen", [KDIM, RPC], dt.float16, kind="ExternalInput")
    Rh = nc.dram_tensor("r_red", [JTILE, NJT * RCOLS], dt.float16,
                        kind="ExternalInput")
    So = nc.dram_tensor("s_out", [RCOLS, RPC], dt.float32,
                        kind="ExternalOutput")

    groups = []
    jt = 0
    while jt < NJT:
        groups.append(list(range(jt, min(jt + GROUP, NJT))))
        jt += GROUP

    with tile.TileContext(nc) as tc:
        with (
            tc.tile_pool(name="const", bufs=1) as cpool,
            tc.tile_pool(name="ksb", bufs=3) as kpool,
            tc.tile_pool(name="ssb", bufs=2) as spool,
            tc.tile_pool(name="d2", bufs=2, space="PSUM") as d2pool,
            tc.tile_pool(name="sacc", bufs=2, space="PSUM") as sapool,
        ):
            a_sb = cpool.tile([KDIM, N], dt.float16)
            b_sb = cpool.tile([KDIM, RPC], dt.float16)
            r_sb = cpool.tile([JTILE, NJT * RCOLS], dt.float16)
            nc.sync.dma_start(out=a_sb, in_=Ah.ap())
            nc.sync.dma_start(out=b_sb, in_=Bh.ap())
            nc.sync.dma_start(out=r_sb, in_=Rh.ap())

            for ib in [i % NIB for i in range(NIB * loop_m)]:
                s_ps = sapool.tile([RCOLS, ICHUNK], dt.float32)
                bcols = b_sb[:, ib * ICHUNK:(ib + 1) * ICHUNK]

                pending = None  # (jts, k_sb) whose reduction is not yet emitted
                for jts in groups:
                    w = len(jts) * ICHUNK
                    d2 = d2pool.tile([JTILE, GROUP * ICHUNK], dt.float32)
                    for idx, jt in enumerate(jts):
                        nc.tensor.matmul(
                            d2[:, idx * ICHUNK:(idx + 1) * ICHUNK],
                            a_sb[:, jt * JTILE:(jt + 1) * JTILE],
                            bcols,
                            start=True, stop=True,
                        )
                    # software pipeline: emit previous group's reductions
                    # between this group's gen and exp so the PE never FIFO
                    # blocks behind a reduction waiting on the ACT engine.
                    if pending is not None and VARIANT == "full":
                        pjts, pk = pending
                        for idx, jt in enumerate(pjts):
                            nc.tensor.matmul(
                                s_ps,
                                r_sb[:, jt * RCOLS:(jt + 1) * RCOLS],
                                pk[:, idx * ICHUNK:(idx + 1) * ICHUNK],
                                start=(jt == 0), stop=(jt == NJT - 1),
                            )
                    if VARIANT != "gen":
                        k_sb = kpool.tile([JTILE, GROUP * ICHUNK], dt.float16)
                        nc.scalar.activation(
                            k_sb[:, :w], d2[:, :w],
                            mybir.ActivationFunctionType.Exp,
                            scale=-1.0 / (2.0 * SIG2),
                        )
                        pending = (jts, k_sb)

                if VARIANT == "full":
                    pjts, pk = pending
                    for idx, jt in enumerate(pjts):
                        nc.tensor.matmul(
                            s_ps,
                            r_sb[:, jt * RCOLS:(jt + 1) * RCOLS],
                            pk[:, idx * ICHUNK:(idx + 1) * ICHUNK],
                            start=(jt == 0), stop=(jt == NJT - 1),
                        )

                s_out = spool.tile([RCOLS, ICHUNK], dt.float32)
                if VARIANT == "full":
                    nc.vector.tensor_copy(s_out, s_ps)
                else:
                    nc.vector.memset(s_out, 0.0)
                nc.sync.dma_start(
                    out=So.ap()[:, ib * ICHUNK:(ib + 1) * ICHUNK], in_=s_out
                )

    nc.compile()
    return nc


def _split_hi_lo(v32: np.ndarray):
    """fp32 -> (hi, lo) float16 pair with v ~= hi + lo."""
    hi = v32.astype(np.float16)
    lo = (v32 - hi.astype(np.float32)).astype(np.float16)
    return hi, lo


def _host_prep(mom: np.ndarray, control_points: np.ndarray):
    x = np.asarray(control_points, np.float32).reshape(N, D)
    p = np.clip(np.asarray(mom, np.float32).reshape(N, D), -1.0, 1.0)

    sq = np.sum(x.astype(np.float64) * x.astype(np.float64), axis=1)
    sq = sq.astype(np.float32)
    xh, xl = _split_hi_lo(x)
    sqh, sql = _split_hi_lo(sq)
    ones = np.ones(N, np.float16)

    # lhsT (stationary, per-j): 13 rows
    A = np.empty((KDIM, N), np.float16)
    A[0:3] = xh.T
    A[3:6] = xl.T
    A[6:9] = xh.T
    A[9] = sqh
    A[10] = sql
    A[11] = ones
    A[12] = ones

    # rhs (moving, per-i): 13 rows
    m2xh = (-2.0 * xh.astype(np.float32)).astype(np.float16)
    m2xl = (-2.0 * xl.astype(np.float32)).astype(np.float16)
    Bfull = np.empty((KDIM, N), np.float16)
    Bfull[0:3] = m2xh.T
    Bfull[3:6] = m2xh.T
    Bfull[6:9] = m2xl.T
    Bfull[9] = ones
    Bfull[10] = ones
    Bfull[11] = sqh
    Bfull[12] = sql

    # reduction matrix R = [p | vec(p (x) x)], packed [128, 64*12]
    R = np.empty((N, RCOLS), np.float32)
    R[:, 0:3] = p
    R[:, 3:12] = (p[:, :, None] * x[:, None, :]).reshape(N, 9)
    Rp = (
        R.reshape(NJT, JTILE, RCOLS)
        .transpose(1, 0, 2)
        .reshape(JTILE, NJT * RCOLS)
        .astype(np.float16)
    )
    return x, p, A, Bfull, Rp


def kernel(mom: np.ndarray, control_points: np.ndarray):
    global last_result
    from concourse.bass_utils import run_bass_kernel_spmd

    x, p, A, Bfull, Rp = _host_prep(mom, control_points)

    loop_m = int(os.environ.get("KERNEL_LOOP_M", "1"))
    key = ("nc", loop_m)
    if key not in _cache:
        _cache[key] = _build_program(loop_m)
    nc = _cache[key]

    in_maps = []
    for c in range(NCORES):
        in_maps.append({
            "a_gen": A,
            "b_gen": np.ascontiguousarray(Bfull[:, c * RPC:(c + 1) * RPC]),
            "r_red": Rp,
        })

    trace = os.environ.get("KERNEL_TRACE", "0") == "1"
    res = run_bass_kernel_spmd(
        nc, in_maps, core_ids=list(range(NCORES)), trace=trace,
    )
    last_result = res

    S = np.concatenate([r["s_out"] for r in res.results], axis=1)  # [12, N]

    dcp = S[0:3].T                                   # [N, 3]
    row = np.einsum("nd,dn->n", p, S[0:3])           # p_i . (K p)_i
    Wx = np.einsum("nd,den->ne", p, S[3:12].reshape(D, D, N))
    dmom = (1.0 / SIG2) * (x * row[:, None] - Wx)

    return (
        dmom.reshape(1, N, D).astype(np.float32),
        dcp.reshape(1, N, D).astype(np.float32),
    )
